# revision 1
# baseline (speedup 1.0000x reference)
"""MultiHeadAttention TRN2 kernel: B=2, S=2048, D=1024, H=16, DK=64, 8 cores.

Sharding: core c handles batch b=c//4 and heads hg=(c%4)*4 .. +3 (data + head
parallel). Projections are column-split by head; out-proj row-split; the
all-reduce after out-proj is done on host (sum of 4 partials per batch).

Device dataflow (per core, all matmuls float32r = full PE rate):
  qT/kT = (wT-slice).T @ QT/KT          -> (128=2 heads, S) per head-pair
  v     = VT.T @ wvT-slice (+ ones col) -> natural (k-rows, 65) chunks
  scoresT[k,q] = kT.T-chunk @ qT        (K=64)
  expT = exp(scoresT/8)                 (ScalarE, the throughput floor)
  ctxU^T[f,q] (+den row) = v_aug.T @ expT  (psum accumulate over k-chunks)
  ctx^T = ctxU^T * (1/den)              (shuffle-broadcast + recip_approx)
  partial_out = ctx^T.T @ woT-slice     -> (S, 1024) partial, summed on host

Attention runs as 8 sweeps over (512-wide q slice, head pair): the scores
tile packs both heads (head hh in cols [512*hh:+512], K=64 contraction in PE
rows [64*hh:+64], concurrent row groups) so one 1024-wide exp serves the pair.

PSUM (8 banks), each stream owns its tags so slot rotations self-chain:
  s_ps0/1: (128,1024) x2 banks each -- scores/exp pipeline          [4 banks]
  cu0/cu1: (128,512)  x1 bank each  -- ctxU accum (2 heads x 1 qvb) [2 banks]
  fp0/fp1: (128,512)  x1 bank each  -- q/k/v projections, out-proj  [2 banks]

Bias handling (exact): bq added on device (per-partition add in qT layout);
bk dropped (softmax shift-invariance); bv and bo folded on host as
out += bv @ wo.T + bo (softmax weights sum to 1).
"""

from contextlib import ExitStack

import numpy as np

B, S, D, H, DK = 2, 2048, 1024, 16, 64
NCORES = 8
HPC = H // (NCORES // B)      # heads per core = 4
R = HPC * DK                  # local feats = 256
NKC = S // 128                # k-chunks = 16
VW = 65                       # v chunk width (64 + ones col)

_CACHE = {}
_LAST_IN_MAPS = None


def _build():
    import concourse.mybir as mybir
    import concourse.tile as tile
    from concourse import bacc

    f32 = mybir.dt.float32
    f32r = mybir.dt.float32r
    Exp = mybir.ActivationFunctionType.Exp
    Add = mybir.AluOpType.add

    nc = bacc.Bacc(
        "TRN2", target_bir_lowering=False, debug=False,
        enable_asserts=True, num_devices=NCORES,
    )

    QT_d = nc.dram_tensor("QT", [D, S], f32r, kind="ExternalInput").ap()
    KT_d = nc.dram_tensor("KT", [D, S], f32r, kind="ExternalInput").ap()
    VT_d = nc.dram_tensor("VT", [D, S], f32r, kind="ExternalInput").ap()
    wqT_d = nc.dram_tensor("wqT", [D, R], f32r, kind="ExternalInput").ap()
    wkT_d = nc.dram_tensor("wkT", [D, R], f32r, kind="ExternalInput").ap()
    wvT_d = nc.dram_tensor("wvT", [D, R], f32r, kind="ExternalInput").ap()
    woT_d = nc.dram_tensor("woT", [R, D], f32r, kind="ExternalInput").ap()
    bq_d = nc.dram_tensor("bq", [R, 1], f32, kind="ExternalInput").ap()
    out_d = nc.dram_tensor("OUT", [S, D], f32, kind="ExternalOutput").ap()

    with tile.TileContext(nc) as tc, ExitStack() as ctx:
        sb = ctx.enter_context(tc.tile_pool(name="sb", bufs=1))
        qkv_in = ctx.enter_context(tc.tile_pool(name="qkv_in", bufs=1))
        expp = ctx.enter_context(tc.tile_pool(name="expp", bufs=7))
        normp = ctx.enter_context(tc.tile_pool(name="normp", bufs=2))
        osb = ctx.enter_context(tc.tile_pool(name="osb", bufs=3))
        psum = ctx.enter_context(tc.tile_pool(name="psum", bufs=1, space="PSUM"))

        cnt = {"s": 0, "f": 0, "c": 0}

        def s_tile():
            i = cnt["s"]; cnt["s"] += 1
            return psum.tile([128, 1024], f32, name=f"s_ps{i % 2}", tag=f"s_ps{i % 2}")

        def f_tile():
            i = cnt["f"]; cnt["f"] += 1
            return psum.tile([128, 512], f32, name=f"fp{i % 2}", tag=f"fp{i % 2}")

        def cu_tile():
            i = cnt["c"]; cnt["c"] += 1
            return psum.tile([128, 512], f32, name=f"cu{i % 2}", tag=f"cu{i % 2}")

        # ---- persistent weights ----
        wq_sb = sb.tile([128, 8 * R], f32r)   # D-chunk d at cols [R*d : R*(d+1)]
        wk_sb = sb.tile([128, 8 * R], f32r)
        wv_sb = sb.tile([128, 8 * R], f32r)
        bq_sb = sb.tile([128, 2], f32)
        for hp in range(2):
            nc.sync.dma_start(bq_sb[:, hp:hp + 1], bq_d[128 * hp:128 * (hp + 1), :])

        # ---- persistent activations ----
        qT_sb = [sb.tile([128, S], f32r, name=f"qT_sb{hp}") for hp in range(2)]
        kT_sb = [sb.tile([128, S], f32r, name=f"kT_sb{hp}") for hp in range(2)]
        v_all = sb.tile([128, HPC * NKC * VW], f32r)  # head h chunk c at cols (h*NKC+c)*VW
        ctxT_sb = [sb.tile([128, S], f32r, name=f"ctxT_sb{cn}") for cn in range(2)]

        onecol = sb.tile([128, 1], f32)
        nc.vector.memset(onecol[:], 1.0)
        vv = v_all.rearrange("p (n c) -> p n c", c=VW)[:, :, 64:65].rearrange(
            "p n c -> p (n c)")
        nc.vector.tensor_copy(vv, onecol[:].broadcast_to((128, HPC * NKC)))

        # ---- loads: one big 3D-AP DMA per half-Sblock (amortizes trigger) ----
        def w_load(w_sb, w_d):
            nc.sync.dma_start(
                w_sb.rearrange("p (d r) -> p d r", d=8),
                w_d.rearrange("(d p) r -> p d r", p=128))

        def big_load(src, sblk, tag, bufs, eng=None):
            eng = eng or nc.sync
            tiles = []
            for hf in range(2):
                t = qkv_in.tile([128, 2048], f32r, name=tag, tag=tag, bufs=bufs)
                eng.dma_start(
                    t.rearrange("p (d s) -> p d s", d=4),
                    src.rearrange("(d p) s -> p d s", p=128)[
                        :, 4 * hf:4 * hf + 4, 512 * sblk:512 * (sblk + 1)])
                tiles.append(t)
            return lambda d: tiles[d // 4][:, 512 * (d % 4):512 * (d % 4 + 1)]

        # ---- projections (dedicated fp tags; stream at DMA pace) ----
        def qk_proj(src, w_sb, dst_sb, sblk, bias):
            ins = big_load(src, sblk, "qk_in", 4)
            for hp in range(2):
                p_ps = f_tile()
                for d in range(8):
                    nc.tensor.matmul(
                        p_ps[:],
                        w_sb[:, R * d + 128 * hp:R * d + 128 * (hp + 1)],
                        ins(d), start=(d == 0), stop=(d == 7))
                dst = dst_sb[hp][:, 512 * sblk:512 * (sblk + 1)]
                if bias:
                    nc.vector.tensor_scalar(
                        dst, p_ps[:], bq_sb[:, hp:hp + 1], None, op0=Add)
                else:
                    nc.vector.tensor_copy(dst, p_ps[:])

        def v_proj(sblk, eng=None):
            ins = big_load(VT_d, sblk, "v_in", 3, eng=eng or nc.scalar)
            for sub in range(4):
                c = 4 * sblk + sub
                v_ps = f_tile()
                for d in range(8):
                    nc.tensor.matmul(
                        v_ps[0:128, 0:R], ins(d)[:, 128 * sub:128 * (sub + 1)],
                        wv_sb[:, R * d:R * (d + 1)], start=(d == 0), stop=(d == 7))
                va = v_all.rearrange("p (h n c) -> p h n c", h=HPC, n=NKC)
                nc.vector.tensor_copy(
                    va[:, :, c:c + 1, 0:64],
                    v_ps[0:128, 0:R].rearrange("p (h n c) -> p h n c", h=HPC, n=1))

        # VT streams on the scalar HWDGE queue in parallel with QT/KT on sync
        nc.scalar.dma_start(
            wv_sb.rearrange("p (d r) -> p d r", d=8),
            wvT_d.rearrange("(d p) r -> p d r", p=128))
        w_load(wq_sb, wqT_d)
        for sblk in range(2):
            v_proj(sblk)
            qk_proj(QT_d, wq_sb, qT_sb, sblk, True)
        for sblk in range(2, 4):
            qk_proj(QT_d, wq_sb, qT_sb, sblk, True)
        w_load(wk_sb, wkT_d)
        qk_proj(KT_d, wk_sb, kT_sb, 0, False)
        qk_proj(KT_d, wk_sb, kT_sb, 1, False)
        qk_proj(KT_d, wk_sb, kT_sb, 2, False)
        v_proj(2, eng=nc.sync)
        qk_proj(KT_d, wk_sb, kT_sb, 3, False)
        v_proj(3, eng=nc.sync)

        # ---- out-proj weights ----
        wo_sb = [sb.tile([128, D], f32r, name=f"wo_sb{cn}") for cn in range(2)]
        for cn in range(2):
            nc.sync.dma_start(wo_sb[cn][:], woT_d[128 * cn:128 * (cn + 1), :])

        def out_proj2(oq, half):
            out_proj(oq, range(4 * oq + 2 * half, 4 * oq + 2 * half + 2))

        def out_proj(oq, scs=None):
            for sc in (scs if scs is not None else range(4 * oq, 4 * oq + 4)):
                o_sb = osb.tile([128, D], f32, name="o_sb")
                for nb in range(2):
                    # the tail slice runs after the last exp: score and ctxU
                    # tags are free, so rotate all three families (6-deep)
                    if oq == 3:
                        k = (2 * sc + nb) % 3
                        o_ps = (f_tile() if k == 0 else
                                s_tile()[:, 0:512] if k == 1 else cu_tile())
                    else:
                        o_ps = f_tile()
                    for cn in range(2):
                        nc.tensor.matmul(
                            o_ps[:],
                            ctxT_sb[cn][:, 128 * sc:128 * (sc + 1)],
                            wo_sb[cn][:, 512 * nb:512 * (nb + 1)],
                            start=(cn == 0), stop=(cn == 1))
                    dst = o_sb[:, 512 * nb:512 * (nb + 1)]
                    # during attention ACT is the bottleneck -> evac on DVE;
                    # the final slice runs after the last exp -> both engines
                    # are idle, so alternate to halve the evac chain
                    if oq == 3 and (2 * sc + nb) % 2 == 0:
                        nc.scalar.copy(dst, o_ps[:])
                    else:
                        nc.vector.tensor_copy(dst, o_ps[:])
                nc.gpsimd.dma_start(out_d[128 * sc:128 * (sc + 1), :], o_sb[:])

        # ---- attention: 8 sweeps of (512-wide q slice, head pair) ----
        # scores tile packs both heads: head hh in cols [512*hh:+512], with
        # its K=64 contraction in PE rows [64*hh:+64] (concurrent row groups).
        # One 1024-wide exp covers the pair.
        for qvb in range(4):
            for hp in range(2):
                c_ps = [cu_tile() for _ in range(2)]   # [hh]
                for c in range(NKC):
                    s_ps = s_tile()
                    for hh in range(2):
                        nc.tensor.matmul(
                            s_ps[:, 512 * hh:512 * (hh + 1)],
                            kT_sb[hp][64 * hh:64 * (hh + 1), 128 * c:128 * (c + 1)],
                            qT_sb[hp][64 * hh:64 * (hh + 1),
                                      512 * qvb:512 * (qvb + 1)],
                            start=True, stop=True)
                    expT = expp.tile([128, 1024], f32r, name="expT")
                    nc.scalar.activation(expT[:], s_ps[:], Exp, scale=0.125)
                    for hh in range(2):
                        gh = 2 * hp + hh
                        nc.tensor.matmul(
                            c_ps[hh][0:VW, :],
                            v_all[:, (gh * NKC + c) * VW:(gh * NKC + c + 1) * VW],
                            expT[:, 512 * hh:512 * (hh + 1)],
                            start=(c == 0), stop=(c == NKC - 1))
                # normalization: drain c_ps with 3 short psum reads (2 shuffles
                # + 1 copy) so the cu slot frees early, then recip + one
                # (64,512) multiply whose SBUF inputs both start at partition 0
                for hh in range(2):
                    rin = normp.tile([64, 512], f32, name="rin")
                    nc.vector.stream_shuffle(rin[0:32, :], c_ps[hh][64:96, :], [0] * 32)
                    nc.vector.stream_shuffle(rin[32:64, :], c_ps[hh][64:96, :], [0] * 32)
                    tmp = normp.tile([64, 512], f32, name="tmp")
                    if qvb == 3 and hp == 1:
                        # last sweep: ACT is idle after the final exp; offload
                        # the evac copy so out_proj(3) ungates sooner
                        nc.scalar.copy(tmp[:], c_ps[hh][0:64, :])
                    else:
                        nc.vector.tensor_copy(tmp[:], c_ps[hh][0:64, :])
                    rb = normp.tile([64, 512], f32, name="rb")
                    nc.vector.reciprocal_approx_fast(out=rb[:], in_=rin[:])
                    nc.vector.tensor_mul(
                        ctxT_sb[hp][64 * hh:64 * (hh + 1),
                                    512 * qvb:512 * (qvb + 1)],
                        tmp[:], rb[:])
                if qvb > 0:
                    out_proj2(qvb - 1, hp)
        out_proj(3)

    nc.compile()
    return nc


def kernel(Q, K, V, wq, bq, wk, bk, wv, bv, wo, bo):
    from concourse.bass_utils import run_bass_kernel_spmd

    if "nc" not in _CACHE:
        _CACHE["nc"] = _build()
    nc = _CACHE["nc"]

    Q = np.asarray(Q, np.float32)
    K = np.asarray(K, np.float32)
    V = np.asarray(V, np.float32)
    QT = [np.ascontiguousarray(Q[b].T) for b in range(B)]
    KT = [np.ascontiguousarray(K[b].T) for b in range(B)]
    VT = [np.ascontiguousarray(V[b].T) for b in range(B)]
    wqT = [np.ascontiguousarray(np.asarray(wq, np.float32)[g * R:(g + 1) * R].T)
           for g in range(4)]
    wkT = [np.ascontiguousarray(np.asarray(wk, np.float32)[g * R:(g + 1) * R].T)
           for g in range(4)]
    wvT = [np.ascontiguousarray(np.asarray(wv, np.float32)[g * R:(g + 1) * R].T)
           for g in range(4)]
    woT = [np.ascontiguousarray(np.asarray(wo, np.float32)[:, g * R:(g + 1) * R].T)
           for g in range(4)]
    bqs = [np.ascontiguousarray(np.asarray(bq, np.float32)[g * R:(g + 1) * R, None])
           for g in range(4)]

    in_maps = []
    for c in range(NCORES):
        b, g = c // 4, c % 4
        in_maps.append({
            "QT": QT[b], "KT": KT[b], "VT": VT[b],
            "wqT": wqT[g], "wkT": wkT[g], "wvT": wvT[g], "woT": woT[g],
            "bq": bqs[g],
        })

    global _LAST_IN_MAPS
    _LAST_IN_MAPS = in_maps
    res = run_bass_kernel_spmd(nc, in_maps, core_ids=list(range(NCORES)))

    host_bias = (np.asarray(bv, np.float32) @ np.asarray(wo, np.float32).T
                 + np.asarray(bo, np.float32))
    out = np.zeros((B, S, D), np.float32)
    for c in range(NCORES):
        out[c // 4] += res.results[c]["OUT"]
    out += host_bias[None, None, :]
    return out



# revision 10
# speedup vs baseline: 1.1946x; 1.1946x over previous
"""MultiHeadAttention TRN2 kernel: B=2, S=2048, D=1024, H=16, DK=64, 8 cores.

Sharding: core c handles batch b=c//4 and heads hg=(c%4)*4 .. +3 (data + head
parallel). Projections are column-split by head; out-proj row-split; the
all-reduce after out-proj is done on host (sum of 4 partials per batch).

All activations/weights stream HBM<->SBUF as bf16 (host converts), halving
DMA on the serial DMA-engine resource. Matmul inputs are bf16 (1 cycle/row at
any moving size) except qT/kT which stay f32r for exp-input precision.

Device dataflow (per core):
  qT/kT = (w-slice).T @ QT/KT      -> [feat 128 (2 heads), seq] f32r, 256-wide
  v     = VT.T @ wv-slice          -> natural [kpos, 4*64] chunks -> v_all bf16
                                      [kpos, head*16*65] with ones col (den)
  scoresT[kpos,q] = kT-chunk.T @ qT  (K=64, both heads packed in one
                                      [128,1024] 2-bank PSUM tile)
  expT = exp(scoresT/8)            -> bf16 SBUF (ACT, the throughput floor)
  ctx[q, 65] += expT-slice.T @ v-chunk   (F=65 bf16, PSUM accum over kpos;
                                          col 64 accumulates the denominator)
  ctx_sb = ctx * recip(den)        -> [q, 128(2 heads)] f32 per qsub
  ctxT = PE-transpose(ctx_sb)      -> ctxT_sb [feat, q] bf16
  out[q, Dout] = ctxT.T @ wo       -> 256-wide chunks -> bf16 partial out

The whole thing is software-pipelined at DMA-chunk granularity: weights ->
K s0 -> Q s0 -> V s0 -> K s1 -> V s1 -> ... arrive on the sync queue while
projections and attention sweeps (qvb, head-pair) consume them; proj tiles,
out-proj, norm and transposes are injected between attention iterations so
ACT runs exp back-to-back from ~12us on.

PSUM (8 banks): scA/scB [128,1024] x2 banks each (scores/exp dbuf) | ctxA,
ctxB [128,455] (7 of 8 per-sweep [128,65] accumulators, parity-alternating) |
ctxC [128,512] (8th accumulator per parity) | projC [128,512] (2 rotating
256-wide slots for q/k/v-proj, out-proj and ctx transposes).

Bias handling (exact): bq added on device (per-partition add in qT layout);
bk dropped (softmax shift-invariance); bv and bo folded on host as
out += bv @ wo.T + bo (softmax weights sum to 1).
"""

from contextlib import ExitStack

import numpy as np

B, S, D, H, DK = 2, 2048, 1024, 16, 64
NCORES = 8
HPC = H // (NCORES // B)      # heads per core = 4
R = HPC * DK                  # local feats = 256
NKC = S // 128                # 128-wide k chunks = 16
VW = 65                       # v chunk width (64 + ones col)

_CACHE = {}
_LAST_IN_MAPS = None


def _build():
    import concourse.mybir as mybir
    import concourse.tile as tile
    from concourse import bacc

    f32 = mybir.dt.float32
    f32r = mybir.dt.float32r
    bf16 = mybir.dt.bfloat16
    Exp = mybir.ActivationFunctionType.Exp
    Add = mybir.AluOpType.add
    Mult = mybir.AluOpType.mult

    nc = bacc.Bacc(
        "TRN2", target_bir_lowering=False, debug=False,
        enable_asserts=True, num_devices=NCORES,
    )

    QT_d = nc.dram_tensor("QT", [D, S], bf16, kind="ExternalInput").ap()
    KT_d = nc.dram_tensor("KT", [D, S], bf16, kind="ExternalInput").ap()
    VT_d = nc.dram_tensor("VT", [D, S], bf16, kind="ExternalInput").ap()
    wqT_d = nc.dram_tensor("wqT", [D, R], bf16, kind="ExternalInput").ap()
    wkT_d = nc.dram_tensor("wkT", [D, R], bf16, kind="ExternalInput").ap()
    wvT_d = nc.dram_tensor("wvT", [D, R], bf16, kind="ExternalInput").ap()
    woT_d = nc.dram_tensor("woT", [R, D], bf16, kind="ExternalInput").ap()
    bq_d = nc.dram_tensor("bq", [R, 1], f32, kind="ExternalInput").ap()
    id_d = nc.dram_tensor("ident", [128, 128], f32, kind="ExternalInput").ap()
    out_d = nc.dram_tensor("OUT", [S, D], bf16, kind="ExternalOutput").ap()

    with tile.TileContext(nc) as tc, ExitStack() as ctx:
        sb = ctx.enter_context(tc.tile_pool(name="sb", bufs=1))
        qin = ctx.enter_context(tc.tile_pool(name="qin", bufs=4))
        kin = ctx.enter_context(tc.tile_pool(name="kin", bufs=4))
        vin = ctx.enter_context(tc.tile_pool(name="vin", bufs=4))
        expp = ctx.enter_context(tc.tile_pool(name="expp", bufs=6))
        cxp = ctx.enter_context(tc.tile_pool(name="cxp", bufs=4))
        osb = ctx.enter_context(tc.tile_pool(name="osb", bufs=3))
        psum = ctx.enter_context(tc.tile_pool(name="psum", bufs=1, space="PSUM"))

        # ---- persistent PSUM containers (8 banks exactly) ----
        # PSUM accumulation groups are zero-region (= bank) granular: a
        # start_tensor_calc matmul zeroes its whole bank, so each bank holds
        # exactly one live group. Scores halves are full banks; the 4 ctx
        # accumulators of one hh live in one bank as a single group; proj /
        # out-proj / transpose rotate through two whole-bank slots.
        sc_ps = [psum.tile([128, 1024], f32, name=f"sc{i}") for i in range(2)]
        ctxH = [psum.tile([128, 260], f32, name=f"ctxh{i}") for i in range(2)]
        projAB = [psum.tile([128, 512], f32, name=f"proj{i}") for i in range(2)]

        cnt = {"p": 0}

        def p_slot(w):
            # rotating whole-bank psum slot for q/k/v-proj, out-proj and
            # transposes; overlapping-view hazards serialize reuse
            i = cnt["p"]; cnt["p"] += 1
            return projAB[i % 2][:, 0:w]

        # ---- persistent SBUF ----
        wq_sb = sb.tile([128, 8 * R], bf16)   # D-chunk d at cols [R*d : R*(d+1)]
        wk_sb = sb.tile([128, 8 * R], bf16)
        wv_sb = sb.tile([128, 8 * R], bf16)
        wo_sb = [sb.tile([128, D], bf16, name=f"wo_sb{cn}") for cn in range(2)]
        bq_sb = sb.tile([128, 2], f32)
        id_sb = sb.tile([128, 128], f32)

        qT_sb = [sb.tile([128, S], f32r, name=f"qT_sb{hp}") for hp in range(2)]
        kT_sb = [sb.tile([128, S], f32r, name=f"kT_sb{hp}") for hp in range(2)]
        v_all = sb.tile([128, HPC * NKC * VW], bf16)  # (h, c) at (h*NKC+c)*VW
        ctxT_sb = [sb.tile([128, S], bf16, name=f"ctxT_sb{cn}") for cn in range(2)]

        onecol = sb.tile([128, 1], f32)
        nc.vector.memset(onecol[:], 1.0)
        vv = v_all.rearrange("p (n c) -> p n c", c=VW)[:, :, 64:65].rearrange(
            "p n c -> p (n c)")
        nc.vector.tensor_copy(vv, onecol[:].broadcast_to((128, HPC * NKC)))

        def w_load(w_sb, w_d):
            nc.sync.dma_start(
                w_sb.rearrange("p (d r) -> p d r", d=8),
                w_d.rearrange("(d p) r -> p d r", p=128))

        # staging tiles: one [128, 2048] bf16 tile covers 4 d-chunks x 512
        # seq; a (tensor, sblk) pair = 2 tiles (d 0-3, d 4-7)
        stage = {}

        def chunk_load(src, pool, tag, sblk):
            tiles = []
            for hf in range(2):
                t = pool.tile([128, 2048], bf16, name=tag, tag=tag)
                nc.sync.dma_start(
                    t.rearrange("p (d s) -> p d s", d=4),
                    src.rearrange("(d p) s -> p d s", p=128)[
                        :, 4 * hf:4 * hf + 4, 512 * sblk:512 * (sblk + 1)])
                tiles.append(t)
            stage[(tag, sblk)] = tiles

        def staged(tag, sblk, d, cols):
            # d-chunk d of sblk, column slice `cols` within the 512-wide sblk
            t = stage[(tag, sblk)][d // 4]
            base = 512 * (d % 4)
            return t[:, base + cols[0]:base + cols[1]]

        # ---- projection tiles ----
        def qk_proj(tag, w_sb, dst_sb, hp, j, bias):
            # (hp, j): 256 seq cols [256j : 256j+256] of head-pair hp
            sblk, half = j // 2, j % 2
            cols = (256 * half, 256 * half + 256)
            p_ps = p_slot(256)
            for d in range(8):
                nc.tensor.matmul(
                    p_ps[:],
                    w_sb[:, R * d + 128 * hp:R * d + 128 * (hp + 1)],
                    staged(tag, sblk, d, cols), start=(d == 0), stop=(d == 7))
            dst = dst_sb[hp][:, 256 * j:256 * (j + 1)]
            if bias:
                nc.vector.tensor_scalar(
                    dst, p_ps[:], bq_sb[:, hp:hp + 1], None, op0=Add)
            else:
                nc.vector.tensor_copy(dst, p_ps[:])

        def v_proj(c):
            # kpos chunk c (128 rows): out [kpos, 256 feats] -> v_all slices
            sblk, sub = c // 4, c % 4
            cols = (128 * sub, 128 * sub + 128)
            v_ps = p_slot(256)
            for d in range(8):
                nc.tensor.matmul(
                    v_ps[:], staged("v", sblk, d, cols),
                    wv_sb[:, R * d:R * (d + 1)], start=(d == 0), stop=(d == 7))
            va = v_all.rearrange("p (h n c) -> p h n c", h=HPC, n=NKC)
            nc.vector.tensor_copy(
                va[:, :, c:c + 1, 0:64],
                v_ps[:].rearrange("p (h n c) -> p h n c", h=HPC, n=1))

        # ---- attention sweep pieces ----
        def sweep_iter(qvb, hp, c, sweep):
            s_ps = sc_ps[c % 2]
            for hh in range(2):
                nc.tensor.matmul(
                    s_ps[:, 512 * hh:512 * (hh + 1)],
                    kT_sb[hp][64 * hh:64 * (hh + 1), 128 * c:128 * (c + 1)],
                    qT_sb[hp][64 * hh:64 * (hh + 1), 512 * qvb:512 * (qvb + 1)],
                    start=True, stop=True)
            expT = expp.tile([128, 1024], bf16, name="expT")
            nc.scalar.activation(expT[:], s_ps[:], Exp, scale=0.125)
            for hh in range(2):
                gh = HPC // 2 * hp + hh
                for qs in range(4):
                    nc.tensor.matmul(
                        ctxH[hh][0:128, 65 * qs:65 * qs + VW],
                        expT[:, 512 * hh + 128 * qs:512 * hh + 128 * (qs + 1)],
                        v_all[:, (gh * NKC + c) * VW:(gh * NKC + c + 1) * VW],
                        start=(c == 0 and qs == 0),
                        stop=(c == NKC - 1 and qs == 3))

        ctx_stage = {}

        def norm(qvb, hp):
            # drain both ctxH banks: per hh one strided recip over the 4
            # denominator columns + one strided multiply into the f32
            # staging tile cs [128, (hh, qs, 64)]
            cs = cxp.tile([128, 512], f32, name="ctxs", tag=f"ctxs{hp}")
            ctx_stage[(qvb, hp)] = cs
            for hh in range(2):
                t3 = ctxH[hh].rearrange("p (qs w) -> p qs w", w=VW)
                rb = cxp.tile([128, 4], f32, name="rb", tag=f"rb{hh}")
                nc.vector.reciprocal_approx_fast(
                    out=rb[:], in_=t3[:, :, 64:65].rearrange("p a b -> p (a b)"))
                nc.vector.tensor_mul(
                    cs.rearrange("p (qs hh f) -> p qs hh f", qs=4, hh=2)[
                        :, :, hh, :],
                    t3[:, :, 0:64],
                    rb.rearrange("p (a b) -> p a b", b=1).broadcast_to(
                        (128, 4, 64)))

        def transpose(qvb, hp, qs):
            cs = ctx_stage[(qvb, hp)]
            lhsT = cs[:, 128 * qs:128 * (qs + 1)]
            tp = p_slot(128)
            nc.tensor.matmul(tp, lhsT, id_sb[:], is_transpose=True)
            nc.vector.tensor_copy(
                ctxT_sb[hp][:, 512 * qvb + 128 * qs:512 * qvb + 128 * (qs + 1)],
                tp)
            if qs == 3:
                del ctx_stage[(qvb, hp)]

        def out_proj(qvb, qs):
            o_sb = osb.tile([128, D], bf16, name="o_sb")
            qcols = (512 * qvb + 128 * qs, 512 * qvb + 128 * (qs + 1))
            for dc in range(4):
                o_ps = p_slot(256)
                for cn in range(2):
                    nc.tensor.matmul(
                        o_ps[:], ctxT_sb[cn][:, qcols[0]:qcols[1]],
                        wo_sb[cn][:, 256 * dc:256 * (dc + 1)],
                        start=(cn == 0), stop=(cn == 1))
                nc.vector.tensor_copy(o_sb[:, 256 * dc:256 * (dc + 1)], o_ps[:])
            nc.gpsimd.dma_start(out_d[qcols[0]:qcols[1], :], o_sb[:])

        # ================= emission schedule =================
        # prologue DMAs (sync queue order = arrival order)
        for hpp in range(2):
            nc.sync.dma_start(bq_sb[:, hpp:hpp + 1],
                              bq_d[128 * hpp:128 * (hpp + 1), :])
        nc.sync.dma_start(id_sb[:], id_d[:, :])
        w_load(wk_sb, wkT_d)
        w_load(wq_sb, wqT_d)
        w_load(wv_sb, wvT_d)
        chunk_load(KT_d, kin, "k", 0)
        chunk_load(QT_d, qin, "q", 0)
        chunk_load(VT_d, vin, "v", 0)

        # prologue compute
        for hp in range(2):
            for j in range(2):
                qk_proj("k", wk_sb, kT_sb, hp, j, False)
        chunk_load(KT_d, kin, "k", 1)
        for hp in range(2):
            for j in range(2):
                qk_proj("q", wq_sb, qT_sb, hp, j, True)
        chunk_load(VT_d, vin, "v", 1)
        for c in range(4):
            v_proj(c)

        # injection plans: per sweep, dict iter -> list of thunks
        def mk_inject():
            return {c: [] for c in range(NKC)}

        inj = {s: mk_inject() for s in range(8)}

        def add(s, c, fn, *a):
            inj[s][c].append((fn, a))

        # sweep 0 = (qvb0, hp0): stream in remaining K, V; proj them
        add(0, 0, chunk_load, KT_d, kin, "k", 2)
        add(0, 0, qk_proj, "k", wk_sb, kT_sb, 0, 2, False)
        add(0, 1, qk_proj, "k", wk_sb, kT_sb, 1, 2, False)
        add(0, 2, chunk_load, VT_d, vin, "v", 2)
        add(0, 2, qk_proj, "k", wk_sb, kT_sb, 0, 3, False)
        add(0, 3, qk_proj, "k", wk_sb, kT_sb, 1, 3, False)
        add(0, 3, v_proj, 4)
        add(0, 4, chunk_load, KT_d, kin, "k", 3)
        add(0, 4, v_proj, 5)
        add(0, 5, qk_proj, "k", wk_sb, kT_sb, 0, 4, False)
        add(0, 5, v_proj, 6)
        add(0, 6, qk_proj, "k", wk_sb, kT_sb, 1, 4, False)
        add(0, 6, v_proj, 7)
        add(0, 7, chunk_load, VT_d, vin, "v", 3)
        add(0, 7, qk_proj, "k", wk_sb, kT_sb, 0, 5, False)
        add(0, 8, qk_proj, "k", wk_sb, kT_sb, 1, 5, False)
        add(0, 8, v_proj, 8)
        add(0, 9, lambda: [nc.sync.dma_start(wo_sb[cn][:],
                                             woT_d[128 * cn:128 * (cn + 1), :])
                           for cn in range(2)])
        add(0, 9, qk_proj, "k", wk_sb, kT_sb, 0, 6, False)
        add(0, 9, v_proj, 9)
        add(0, 10, qk_proj, "k", wk_sb, kT_sb, 1, 6, False)
        add(0, 10, v_proj, 10)
        add(0, 11, chunk_load, QT_d, qin, "q", 1)
        add(0, 11, qk_proj, "k", wk_sb, kT_sb, 0, 7, False)
        add(0, 11, v_proj, 11)
        add(0, 12, qk_proj, "k", wk_sb, kT_sb, 1, 7, False)
        add(0, 12, v_proj, 12)
        add(0, 13, v_proj, 13)
        add(0, 14, v_proj, 14)
        add(0, 15, v_proj, 15)

        # sweep 1 = (qvb0, hp1): qproj qvb1; norms/transposes of sweep 0
        for i, c in enumerate((4, 6, 8, 10)):
            add(1, c, qk_proj, "q", wq_sb, qT_sb, i % 2, 2 + i // 2, True)
        add(1, 11, chunk_load, QT_d, qin, "q", 2)
        # sweep 3: qproj qvb3
        for i, c in enumerate((4, 6, 8, 10)):
            add(3, c, qk_proj, "q", wq_sb, qT_sb, i % 2, 4 + i // 2, True)
        add(3, 11, chunk_load, QT_d, qin, "q", 3)
        for i, c in enumerate((4, 6, 8, 10)):
            add(5, c, qk_proj, "q", wq_sb, qT_sb, i % 2, 6 + i // 2, True)

        # run the 8 sweeps; norms drain the ctx banks right after each sweep
        # (the next sweep's first ctx matmul zero-starts the same banks)
        for sweep in range(8):
            qvb, hp = sweep // 2, sweep % 2
            for c in range(NKC):
                for fn, a in inj[sweep][c]:
                    fn(*a)
                if hp == 1 and c in (1, 3, 5, 7):
                    # transposes for the hp0 sweep of this qvb
                    transpose(qvb, 0, c // 2)
                if hp == 0 and qvb > 0 and c < 8 and c % 2 == 0:
                    # out-proj of the previous qvb, one qsub chain per 2 iters
                    out_proj(qvb - 1, c // 2)
                sweep_iter(qvb, hp, c, sweep)
            norm(qvb, hp)
            if hp == 1:
                for qs in range(4):
                    transpose(qvb, 1, qs)
        for qs in range(4):
            out_proj(3, qs)

    nc.compile()
    return nc


def kernel(Q, K, V, wq, bq, wk, bk, wv, bv, wo, bo):
    import ml_dtypes
    from concourse.bass_utils import run_bass_kernel_spmd

    if "nc" not in _CACHE:
        _CACHE["nc"] = _build()
    nc = _CACHE["nc"]

    bf = ml_dtypes.bfloat16
    Q = np.asarray(Q, np.float32)
    K = np.asarray(K, np.float32)
    V = np.asarray(V, np.float32)
    QT = [np.ascontiguousarray(Q[b].T).astype(bf) for b in range(B)]
    KT = [np.ascontiguousarray(K[b].T).astype(bf) for b in range(B)]
    VT = [np.ascontiguousarray(V[b].T).astype(bf) for b in range(B)]
    wqT = [np.ascontiguousarray(np.asarray(wq, np.float32)[g * R:(g + 1) * R].T
                                ).astype(bf) for g in range(4)]
    wkT = [np.ascontiguousarray(np.asarray(wk, np.float32)[g * R:(g + 1) * R].T
                                ).astype(bf) for g in range(4)]
    wvT = [np.ascontiguousarray(np.asarray(wv, np.float32)[g * R:(g + 1) * R].T
                                ).astype(bf) for g in range(4)]
    woT = [np.ascontiguousarray(np.asarray(wo, np.float32)[:, g * R:(g + 1) * R].T
                                ).astype(bf) for g in range(4)]
    bqs = [np.ascontiguousarray(np.asarray(bq, np.float32)[g * R:(g + 1) * R, None])
           for g in range(4)]
    ident = np.eye(128, dtype=np.float32)

    in_maps = []
    for c in range(NCORES):
        b, g = c // 4, c % 4
        in_maps.append({
            "QT": QT[b], "KT": KT[b], "VT": VT[b],
            "wqT": wqT[g], "wkT": wkT[g], "wvT": wvT[g], "woT": woT[g],
            "bq": bqs[g], "ident": ident,
        })

    global _LAST_IN_MAPS
    _LAST_IN_MAPS = in_maps
    res = run_bass_kernel_spmd(nc, in_maps, core_ids=list(range(NCORES)))

    host_bias = (np.asarray(bv, np.float32) @ np.asarray(wo, np.float32).T
                 + np.asarray(bo, np.float32))
    out = np.zeros((B, S, D), np.float32)
    for c in range(NCORES):
        out[c // 4] += np.asarray(res.results[c]["OUT"], np.float32)
    out += host_bias[None, None, :]
    return out


# revision 17
# speedup vs baseline: 1.2991x; 1.0875x over previous
"""MultiHeadAttention TRN2 kernel: B=2, S=2048, D=1024, H=16, DK=64, 8 cores.

Sharding: core c handles batch b=c//4 and heads hg=(c%4)*4 .. +3 (data + head
parallel). Projections are column-split by head; out-proj row-split; the
all-reduce after out-proj is done on host (sum of 4 partials per batch).

All activations/weights stream HBM<->SBUF as bf16 (host converts), halving
DMA on the serial DMA-engine resource. Matmul inputs are bf16 (1 cycle/row at
any moving size) except qT/kT which stay f32r for exp-input precision.

Device dataflow (per core):
  qT/kT = (w-slice).T @ QT/KT      -> [feat 128 (2 heads), seq] f32r, 256-wide
  v     = VT.T @ wv-slice          -> natural [kpos, 4*64] chunks -> v_all bf16
                                      [kpos, head*16*65] with ones col (den)
  scoresT[kpos,q] = kT-chunk.T @ qT  (K=64, both heads packed in one
                                      [128,1024] 2-bank PSUM tile)
  expT = exp(scoresT/8)            -> bf16 SBUF (ACT, the throughput floor)
  ctx[q, 65] += expT-slice.T @ v-chunk   (F=65 bf16, PSUM accum over kpos;
                                          col 64 accumulates the denominator)
  ctx_sb = ctx * recip(den)        -> [q, 128(2 heads)] f32 per qsub
  ctxT = PE-transpose(ctx_sb)      -> ctxT_sb [feat, q] bf16
  out[q, Dout] = ctxT.T @ wo       -> 256-wide chunks -> bf16 partial out

The whole thing is software-pipelined at DMA-chunk granularity: weights ->
K s0 -> Q s0 -> V s0 -> K s1 -> V s1 -> ... arrive on the sync queue while
projections and attention sweeps (qvb, head-pair) consume them; proj tiles,
out-proj, norm and transposes are injected between attention iterations so
ACT runs exp back-to-back from ~12us on.

PSUM (8 banks): scA/scB [128,1024] x2 banks each (scores/exp dbuf) | ctxA,
ctxB [128,455] (7 of 8 per-sweep [128,65] accumulators, parity-alternating) |
ctxC [128,512] (8th accumulator per parity) | projC [128,512] (2 rotating
256-wide slots for q/k/v-proj, out-proj and ctx transposes).

Bias handling (exact): bq added on device (per-partition add in qT layout);
bk dropped (softmax shift-invariance); bv and bo folded on host as
out += bv @ wo.T + bo (softmax weights sum to 1).
"""

from contextlib import ExitStack

import numpy as np

B, S, D, H, DK = 2, 2048, 1024, 16, 64
NCORES = 8
HPC = H // (NCORES // B)      # heads per core = 4
R = HPC * DK                  # local feats = 256
NKC = S // 128                # 128-wide k chunks = 16
VW = 65                       # v chunk width (64 + ones col)

_CACHE = {}
_LAST_IN_MAPS = None


def _build():
    import concourse.mybir as mybir
    import concourse.tile as tile
    from concourse import bacc

    f32 = mybir.dt.float32
    f32r = mybir.dt.float32r
    bf16 = mybir.dt.bfloat16
    Exp = mybir.ActivationFunctionType.Exp
    Add = mybir.AluOpType.add
    Mult = mybir.AluOpType.mult

    nc = bacc.Bacc(
        "TRN2", target_bir_lowering=False, debug=False,
        enable_asserts=True, num_devices=NCORES,
    )

    QT_d = nc.dram_tensor("QT", [D, S], bf16, kind="ExternalInput").ap()
    KT_d = nc.dram_tensor("KT", [D, S], bf16, kind="ExternalInput").ap()
    VT_d = nc.dram_tensor("VT", [D, S], bf16, kind="ExternalInput").ap()
    wqT_d = nc.dram_tensor("wqT", [D, R], bf16, kind="ExternalInput").ap()
    wkT_d = nc.dram_tensor("wkT", [D, R], bf16, kind="ExternalInput").ap()
    wvT_d = nc.dram_tensor("wvT", [D, R], bf16, kind="ExternalInput").ap()
    woT_d = nc.dram_tensor("woT", [R, D], bf16, kind="ExternalInput").ap()
    bq_d = nc.dram_tensor("bq", [R, 1], f32, kind="ExternalInput").ap()
    id_d = nc.dram_tensor("ident", [128, 128], f32, kind="ExternalInput").ap()
    out_d = nc.dram_tensor("OUT", [S, D], bf16, kind="ExternalOutput").ap()

    with tile.TileContext(nc) as tc, ExitStack() as ctx:
        sb = ctx.enter_context(tc.tile_pool(name="sb", bufs=1))
        qin = ctx.enter_context(tc.tile_pool(name="qin", bufs=4))
        kin = ctx.enter_context(tc.tile_pool(name="kin", bufs=4))
        vin = ctx.enter_context(tc.tile_pool(name="vin", bufs=4))
        expp = ctx.enter_context(tc.tile_pool(name="expp", bufs=14))
        cxp = ctx.enter_context(tc.tile_pool(name="cxp", bufs=4))
        osb = ctx.enter_context(tc.tile_pool(name="osb", bufs=3))
        psum = ctx.enter_context(tc.tile_pool(name="psum", bufs=1, space="PSUM"))

        # ---- persistent PSUM containers (8 banks exactly) ----
        # PSUM accumulation groups are zero-region (= bank) granular: a
        # start_tensor_calc matmul zeroes its whole bank, so each bank holds
        # exactly one live group. Scores halves are full banks; the 4 ctx
        # accumulators of one hh live in one bank as a single group; proj /
        # out-proj / transpose rotate through two whole-bank slots.
        sc_ps = [psum.tile([128, 1024], f32, name=f"sc{i}") for i in range(2)]
        ctxH = [psum.tile([128, 260], f32, name=f"ctxh{i}") for i in range(2)]
        projAB = [psum.tile([128, 512], f32, name=f"proj{i}") for i in range(2)]

        cnt = {"p": 0}

        def p_slot(w):
            # rotating whole-bank psum slot for q/k/v-proj, out-proj and
            # transposes; overlapping-view hazards serialize reuse
            i = cnt["p"]; cnt["p"] += 1
            return projAB[i % 2][:, 0:w]

        # ---- persistent SBUF ----
        wq_sb = sb.tile([128, 8 * R], bf16)   # D-chunk d at cols [R*d : R*(d+1)]
        wk_sb = sb.tile([128, 8 * R], bf16)
        wv_sb = sb.tile([128, 8 * R], bf16)
        wo_sb = [sb.tile([128, D], bf16, name=f"wo_sb{cn}") for cn in range(2)]
        bq_sb = sb.tile([128, 2], f32)
        id_sb = sb.tile([128, 128], f32)

        qT_sb = [sb.tile([128, S], f32r, name=f"qT_sb{hp}") for hp in range(2)]
        kT_sb = [sb.tile([128, S], f32r, name=f"kT_sb{hp}") for hp in range(2)]
        v_all = sb.tile([128, HPC * NKC * VW], bf16)  # (h, c) at (h*NKC+c)*VW
        ctxT_sb = [sb.tile([128, S], bf16, name=f"ctxT_sb{cn}") for cn in range(2)]

        onecol = sb.tile([128, 1], f32)
        nc.vector.memset(onecol[:], 1.0)
        vv = v_all.rearrange("p (n c) -> p n c", c=VW)[:, :, 64:65].rearrange(
            "p n c -> p (n c)")
        nc.vector.tensor_copy(vv, onecol[:].broadcast_to((128, HPC * NKC)))

        def w_load(w_sb, w_d):
            nc.sync.dma_start(
                w_sb.rearrange("p (d r) -> p d r", d=8),
                w_d.rearrange("(d p) r -> p d r", p=128))

        # staging tiles: one [128, 2048] bf16 tile covers 4 d-chunks x 512
        # seq; a (tensor, sblk) pair = 2 tiles (d 0-3, d 4-7)
        stage = {}

        def chunk_load(src, pool, tag, sblk):
            tiles = []
            for hf in range(2):
                t = pool.tile([128, 2048], bf16, name=tag, tag=tag)
                nc.sync.dma_start(
                    t.rearrange("p (d s) -> p d s", d=4),
                    src.rearrange("(d p) s -> p d s", p=128)[
                        :, 4 * hf:4 * hf + 4, 512 * sblk:512 * (sblk + 1)])
                tiles.append(t)
            stage[(tag, sblk)] = tiles

        def staged(tag, sblk, d, cols):
            # d-chunk d of sblk, column slice `cols` within the 512-wide sblk
            t = stage[(tag, sblk)][d // 4]
            base = 512 * (d % 4)
            return t[:, base + cols[0]:base + cols[1]]

        # ---- projection tiles ----
        def qk_proj(tag, w_sb, dst_sb, hp, sblk, bias):
            # (hp, sblk): 512 seq cols of head-pair hp, full-bank psum
            p_ps = p_slot(512)
            for d in range(8):
                nc.tensor.matmul(
                    p_ps[:],
                    w_sb[:, R * d + 128 * hp:R * d + 128 * (hp + 1)],
                    staged(tag, sblk, d, (0, 512)), start=(d == 0), stop=(d == 7))
            dst = dst_sb[hp][:, 512 * sblk:512 * (sblk + 1)]
            if bias:
                nc.vector.tensor_scalar(
                    dst, p_ps[:], bq_sb[:, hp:hp + 1], None, op0=Add)
            else:
                nc.vector.tensor_copy(dst, p_ps[:])

        def v_proj(c):
            # kpos chunk c (128 rows): out [kpos, 256 feats] -> v_all slices
            sblk, sub = c // 4, c % 4
            cols = (128 * sub, 128 * sub + 128)
            v_ps = p_slot(256)
            for d in range(8):
                nc.tensor.matmul(
                    v_ps[:], staged("v", sblk, d, cols),
                    wv_sb[:, R * d:R * (d + 1)], start=(d == 0), stop=(d == 7))
            va = v_all.rearrange("p (h n c) -> p h n c", h=HPC, n=NKC)
            nc.vector.tensor_copy(
                va[:, :, c:c + 1, 0:64],
                v_ps[:].rearrange("p (h n c) -> p h n c", h=HPC, n=1))

        # ---- attention sweep pieces (scores/exp stream + trailing ctx) ----
        exp_ring = {}

        def scores_exp(sweep, c):
            qvb, hp = sweep // 2, sweep % 2
            s_ps = sc_ps[c % 2]
            for hh in range(2):
                nc.tensor.matmul(
                    s_ps[:, 512 * hh:512 * (hh + 1)],
                    kT_sb[hp][64 * hh:64 * (hh + 1), 128 * c:128 * (c + 1)],
                    qT_sb[hp][64 * hh:64 * (hh + 1), 512 * qvb:512 * (qvb + 1)],
                    start=True, stop=True)
            expT = expp.tile([128, 1024], bf16, name="expT")
            nc.scalar.activation(expT[:], s_ps[:], Exp, scale=0.125)
            exp_ring[(sweep, c)] = expT

        def ctx_mm(sweep, c):
            hp = sweep % 2
            expT = exp_ring.pop((sweep, c))
            for hh in range(2):
                gh = HPC // 2 * hp + hh
                for qs in range(4):
                    nc.tensor.matmul(
                        ctxH[hh][0:128, 65 * qs:65 * qs + VW],
                        expT[:, 512 * hh + 128 * qs:512 * hh + 128 * (qs + 1)],
                        v_all[:, (gh * NKC + c) * VW:(gh * NKC + c + 1) * VW],
                        start=(c == 0 and qs == 0),
                        stop=(c == NKC - 1 and qs == 3))

        ctx_stage = {}

        def norm(qvb, hp):
            # drain both ctxH banks: per hh one strided recip over the 4
            # denominator columns + one strided multiply into the f32
            # staging tile cs [128, (hh, qs, 64)]
            cs = cxp.tile([128, 512], f32, name="ctxs", tag=f"ctxs{hp}")
            ctx_stage[(qvb, hp)] = cs
            for hh in range(2):
                t3 = ctxH[hh].rearrange("p (qs w) -> p qs w", w=VW)
                rb = cxp.tile([128, 4], f32, name="rb", tag=f"rb{hh}")
                nc.vector.reciprocal_approx_fast(
                    out=rb[:], in_=t3[:, :, 64:65].rearrange("p a b -> p (a b)"))
                nc.vector.tensor_mul(
                    cs.rearrange("p (qs hh f) -> p qs hh f", qs=4, hh=2)[
                        :, :, hh, :],
                    t3[:, :, 0:64],
                    rb.rearrange("p (a b) -> p a b", b=1).broadcast_to(
                        (128, 4, 64)))

        def transpose(qvb, hp, qs, tail=False):
            cs = ctx_stage[(qvb, hp)]
            lhsT = cs[:, 128 * qs:128 * (qs + 1)]
            tp = p_slot(128)
            nc.tensor.matmul(tp, lhsT, id_sb[:], is_transpose=True)
            # after the last exp ACT is idle; split evacs across ACT and DVE
            dst = ctxT_sb[hp][:, 512 * qvb + 128 * qs:512 * qvb + 128 * (qs + 1)]
            if tail and qs % 2 == 0:
                nc.scalar.copy(dst, tp)
            else:
                nc.vector.tensor_copy(dst, tp)
            if qs == 3:
                del ctx_stage[(qvb, hp)]

        def out_proj(qvb, qs, tail=False):
            o_sb = osb.tile([128, D], bf16, name="o_sb")
            qcols = (512 * qvb + 128 * qs, 512 * qvb + 128 * (qs + 1))
            for dc in range(4):
                o_ps = p_slot(256)
                for cn in range(2):
                    nc.tensor.matmul(
                        o_ps[:], ctxT_sb[cn][:, qcols[0]:qcols[1]],
                        wo_sb[cn][:, 256 * dc:256 * (dc + 1)],
                        start=(cn == 0), stop=(cn == 1))
                if tail and dc % 2 == 0:
                    nc.scalar.copy(o_sb[:, 256 * dc:256 * (dc + 1)], o_ps[:])
                else:
                    nc.vector.tensor_copy(o_sb[:, 256 * dc:256 * (dc + 1)],
                                          o_ps[:])
            nc.gpsimd.dma_start(out_d[qcols[0]:qcols[1], :], o_sb[:])

        # ================= emission schedule =================
        # One global stream of 128 exp units (sweep-major, kchunk-minor);
        # ctx matmuls trail by L units so V arrivals and per-sweep drains
        # never block the scores->exp stream on the in-order PE.
        L = 11
        UNITS = [(s, c) for s in range(8) for c in range(NKC)]
        pre = {u: [] for u in range(len(UNITS) + L)}

        def at(u, fn, *a):
            pre[u].append((fn, a))

        def wo_load():
            for cn in range(2):
                nc.sync.dma_start(wo_sb[cn][:], woT_d[128 * cn:128 * (cn + 1), :])

        # prologue DMAs (sync queue order = arrival order): all K before V
        for hpp in range(2):
            nc.sync.dma_start(bq_sb[:, hpp:hpp + 1],
                              bq_d[128 * hpp:128 * (hpp + 1), :])
        nc.sync.dma_start(id_sb[:], id_d[:, :])
        w_load(wk_sb, wkT_d)
        w_load(wq_sb, wqT_d)
        chunk_load(KT_d, kin, "k", 0)
        chunk_load(QT_d, qin, "q", 0)
        w_load(wv_sb, wvT_d)
        chunk_load(KT_d, kin, "k", 1)
        qk_proj("k", wk_sb, kT_sb, 0, 0, False)
        qk_proj("q", wq_sb, qT_sb, 0, 0, True)

        at(1, qk_proj, "k", wk_sb, kT_sb, 1, 0, False)
        at(2, chunk_load, KT_d, kin, "k", 2)
        at(2, qk_proj, "q", wq_sb, qT_sb, 1, 0, True)
        at(3, chunk_load, KT_d, kin, "k", 3)
        at(3, qk_proj, "k", wk_sb, kT_sb, 0, 1, False)
        at(4, qk_proj, "k", wk_sb, kT_sb, 1, 1, False)
        at(4, chunk_load, VT_d, vin, "v", 0)
        at(5, chunk_load, VT_d, vin, "v", 1)
        at(6, qk_proj, "k", wk_sb, kT_sb, 0, 2, False)
        at(7, qk_proj, "k", wk_sb, kT_sb, 1, 2, False)
        at(7, chunk_load, VT_d, vin, "v", 2)
        at(9, qk_proj, "k", wk_sb, kT_sb, 0, 3, False)
        at(9, chunk_load, VT_d, vin, "v", 3)
        at(10, qk_proj, "k", wk_sb, kT_sb, 1, 3, False)
        at(10, wo_load)
        at(12, chunk_load, QT_d, qin, "q", 1)
        at(24, qk_proj, "q", wq_sb, qT_sb, 0, 1, True)
        at(26, qk_proj, "q", wq_sb, qT_sb, 1, 1, True)
        at(27, chunk_load, QT_d, qin, "q", 2)
        at(56, qk_proj, "q", wq_sb, qT_sb, 0, 2, True)
        at(58, qk_proj, "q", wq_sb, qT_sb, 1, 2, True)
        at(59, chunk_load, QT_d, qin, "q", 3)
        at(88, qk_proj, "q", wq_sb, qT_sb, 0, 3, True)
        at(90, qk_proj, "q", wq_sb, qT_sb, 1, 3, True)
        # out-proj of qvb lands after its hp1 sweep drains (unit 32qvb+42+)
        for qvb in range(3):
            for qs in range(4):
                at(32 * qvb + 44 + 2 * qs, out_proj, qvb, qs, False)

        for u in range(len(UNITS) + L):
            for fn, a in pre[u]:
                fn(*a)
            if u < len(UNITS):
                scores_exp(*UNITS[u])
            if u >= L:
                s2, c2 = UNITS[u - L]
                if s2 == 0:
                    v_proj(c2)
                ctx_mm(s2, c2)
                if c2 == NKC - 1:
                    qvb2, hp2 = s2 // 2, s2 % 2
                    norm(qvb2, hp2)
                    for qs in range(4):
                        transpose(qvb2, hp2, qs, tail=(s2 == 7))
        for qs in range(4):
            out_proj(3, qs, tail=True)

    nc.compile()
    return nc


def kernel(Q, K, V, wq, bq, wk, bk, wv, bv, wo, bo):
    import ml_dtypes
    from concourse.bass_utils import run_bass_kernel_spmd

    if "nc" not in _CACHE:
        _CACHE["nc"] = _build()
    nc = _CACHE["nc"]

    bf = ml_dtypes.bfloat16
    Q = np.asarray(Q, np.float32)
    K = np.asarray(K, np.float32)
    V = np.asarray(V, np.float32)
    QT = [np.ascontiguousarray(Q[b].T).astype(bf) for b in range(B)]
    KT = [np.ascontiguousarray(K[b].T).astype(bf) for b in range(B)]
    VT = [np.ascontiguousarray(V[b].T).astype(bf) for b in range(B)]
    wqT = [np.ascontiguousarray(np.asarray(wq, np.float32)[g * R:(g + 1) * R].T
                                ).astype(bf) for g in range(4)]
    wkT = [np.ascontiguousarray(np.asarray(wk, np.float32)[g * R:(g + 1) * R].T
                                ).astype(bf) for g in range(4)]
    wvT = [np.ascontiguousarray(np.asarray(wv, np.float32)[g * R:(g + 1) * R].T
                                ).astype(bf) for g in range(4)]
    woT = [np.ascontiguousarray(np.asarray(wo, np.float32)[:, g * R:(g + 1) * R].T
                                ).astype(bf) for g in range(4)]
    bqs = [np.ascontiguousarray(np.asarray(bq, np.float32)[g * R:(g + 1) * R, None])
           for g in range(4)]
    ident = np.eye(128, dtype=np.float32)

    in_maps = []
    for c in range(NCORES):
        b, g = c // 4, c % 4
        in_maps.append({
            "QT": QT[b], "KT": KT[b], "VT": VT[b],
            "wqT": wqT[g], "wkT": wkT[g], "wvT": wvT[g], "woT": woT[g],
            "bq": bqs[g], "ident": ident,
        })

    global _LAST_IN_MAPS
    _LAST_IN_MAPS = in_maps
    res = run_bass_kernel_spmd(nc, in_maps, core_ids=list(range(NCORES)))

    host_bias = (np.asarray(bv, np.float32) @ np.asarray(wo, np.float32).T
                 + np.asarray(bo, np.float32))
    out = np.zeros((B, S, D), np.float32)
    for c in range(NCORES):
        out[c // 4] += np.asarray(res.results[c]["OUT"], np.float32)
    out += host_bias[None, None, :]
    return out
